# revision 16
# baseline (speedup 1.0000x reference)
"""GCN-LSTM fused kernel for Trainium2, 8 NeuronCores.

Strategy (sharding):
  Phase A+B (GCN conv): batch-sharded. Core b owns batch b. It computes
  XW = x[b] @ W^T into its DRAM, then processes ALL edges for its batch:
  dma_gather of XW rows by edge src (edge-major layout), a one-hot
  scatter matrix per 128-edge chunk built on DVE (iota==offset)*norm,
  and a PE matmul accumulating messages per 128-dst-node PSUM window
  (edges are pre-sorted by dst on host; self-loops are folded in as
  extra edges with weight dis^2). No cross-core traffic at all.
  Phase reshard: x2 = memory-reinterpretation of y ([64,10000] flat
  view). AllToAll moves node-slices so each core gets its 1250 nodes
  for every timestep.
  Phase C (LSTM over B=8 steps + FC): node-sharded, core c owns nodes
  [1250c, 1250c+1250). Gates via PE matmuls (bias folded via ones-row),
  nonlinearities on ACT, elementwise on DVE node-major, h transposed
  each step on PE for the recurrence.

Output y[b,n,:] is identical across b (reference broadcasts), so each
core returns its [1250, 64] node rows; host assembles + broadcasts.
"""

import numpy as np

import concourse.bass as bass
import concourse.bacc as bacc
import concourse.mybir as mybir
import concourse.tile as tile
from concourse.bass_utils import run_bass_kernel_spmd

F32 = mybir.dt.float32
I16 = mybir.dt.int16

B, N, S, H, E = 8, 10000, 64, 8, 160000
NCORES = 8
WIN = 128                      # dst-node window (psum partition dim)
NWIN = (N + WIN - 1) // WIN    # 79
GCHUNK = 1024                  # gather idxs per dma_gather (ucode ring cap)
NODES_PER_CORE = N // NCORES   # 1250
NODE_PAD = 1280                # padded to 10 chunks of 128
NCHUNK_NODE = NODE_PAD // 128  # 10

_CACHE = {}


def _prep_edges(edge_index, edge_weight):
    """Sort edges (+self-loops) by dst, pad to the chunk grid.

    Returns gather idxs (wrapped int16), per-chunk-col offsets/norms
    ([128, nch] f32), per-chunk window id, and dis for the host. All
    fp32 math here is O(E) index/scalar prep; the O(E*S) work is on
    device.
    """
    src = edge_index[0].astype(np.int64)
    dst = edge_index[1].astype(np.int64)
    ew = edge_weight.astype(np.float32)

    deg = np.zeros(N, np.float32)
    np.add.at(deg, dst, ew)
    deg += np.float32(1.0)
    dis = (1.0 / np.sqrt(deg)).astype(np.float32)
    norm = (dis[src] * dis[dst] * ew).astype(np.float32)

    # self loops: weight dis^2 at (n, n)
    s_all = np.concatenate([src, np.arange(N, dtype=np.int64)])
    d_all = np.concatenate([dst, np.arange(N, dtype=np.int64)])
    n_all = np.concatenate([norm, (dis * dis).astype(np.float32)])

    order = np.argsort(d_all, kind="stable")
    s_all, d_all, n_all = s_all[order], d_all[order], n_all[order]

    win = d_all // WIN
    counts = np.bincount(win, minlength=NWIN)
    padded = ((counts + 127) // 128) * 128
    total = int(padded.sum())
    total_g = ((total + GCHUNK - 1) // GCHUNK) * GCHUNK

    g_idx = np.zeros(total_g, np.int16)
    off = np.zeros(total_g, np.float32)
    nrm = np.zeros(total_g, np.float32)
    # scatter sorted edges into padded slots
    starts = np.zeros(NWIN + 1, np.int64)
    starts[1:] = np.cumsum(padded)
    csrc = np.zeros(NWIN + 1, np.int64)
    csrc[1:] = np.cumsum(counts)
    pos = starts[win] + (np.arange(len(d_all)) - csrc[win])
    g_idx[pos] = s_all.astype(np.int16)
    off[pos] = (d_all - win * WIN).astype(np.float32)
    nrm[pos] = n_all

    nch = total_g // 128
    chunk_win = np.minimum(starts[1:].searchsorted(np.arange(nch) * 128, side="right"),
                           NWIN - 1)
    # wrapped idx layout: [16, n/16] replicated to 128 partitions
    w = g_idx.reshape(-1, 16).T.astype(np.int16)
    gidx_w = np.tile(w, (8, 1))
    off_p = np.ascontiguousarray(off.reshape(nch, 128).T)
    nrm_p = np.ascontiguousarray(nrm.reshape(nch, 128).T)
    return gidx_w, off_p, nrm_p, chunk_win.astype(np.int64), total_g


def _build(nch, ngather, chunk_win):
    """Build the SPMD Bass/Tile program (identical on all cores)."""
    import os
    phase = os.environ.get("KPHASE", "full")  # a | ab | abr | full
    nc = bacc.Bacc("TRN2", target_bir_lowering=False)

    x_p = nc.declare_dram_parameter("x", [N, S], F32, isOutput=False)
    wt_p = nc.declare_dram_parameter("wt", [S, S], F32, isOutput=False)
    brep_p = nc.declare_dram_parameter("brep", [128, S], F32, isOutput=False)
    gidx_p = nc.declare_dram_parameter("gidx", [128, ngather * GCHUNK // 16],
                                       I16, isOutput=False)
    off_p = nc.declare_dram_parameter("offp", [128, nch], F32, isOutput=False)
    nrm_p = nc.declare_dram_parameter("nrmp", [128, nch], F32, isOutput=False)
    wih_p = nc.declare_dram_parameter("wih", [S + 1, 4 * H], F32, isOutput=False)
    whh_p = nc.declare_dram_parameter("whh", [H, 4 * H], F32, isOutput=False)
    wfc_p = nc.declare_dram_parameter("wfc", [B, S], F32, isOutput=False)
    bfcrep_p = nc.declare_dram_parameter("bfcrep", [128, S], F32, isOutput=False)
    out_p = nc.declare_dram_parameter("out", [NODE_PAD, S], F32, isOutput=True)

    idx_cols = GCHUNK // 16

    with tile.TileContext(nc) as tc:
        with (
            tc.tile_pool(name="const", bufs=1) as cpool,
            tc.tile_pool(name="dram", bufs=1, space="DRAM") as dpool,
        ):
            # --- constants ---
            iota_row = cpool.tile([128, 128], F32)
            nc.gpsimd.iota(iota_row[:], pattern=[[1, 128]], base=0,
                           channel_multiplier=0,
                           allow_small_or_imprecise_dtypes=True)
            iota_col = cpool.tile([128, 1], F32)
            nc.gpsimd.iota(iota_col[:], pattern=[[1, 1]], base=0,
                           channel_multiplier=1,
                           allow_small_or_imprecise_dtypes=True)
            ident = cpool.tile([128, 128], F32)
            nc.vector.tensor_scalar(ident[:], iota_row[:], iota_col[:], None,
                                    op0=mybir.AluOpType.is_equal)
            zb = cpool.tile([128, 1], F32)
            nc.vector.memset(zb[:], 0.0)
            wt_s = cpool.tile([S, S], F32)
            nc.sync.dma_start(wt_s[:], wt_p[:])
            brep_s = cpool.tile([128, S], F32)
            nc.sync.dma_start(brep_s[:], brep_p[:])
            gidx_s = cpool.tile([128, ngather * idx_cols], I16)
            nc.sync.dma_start(gidx_s[:], gidx_p[:])
            off_s = cpool.tile([128, nch], F32)
            nc.sync.dma_start(off_s[:], off_p[:])
            nrm_s = cpool.tile([128, nch], F32)
            nc.sync.dma_start(nrm_s[:], nrm_p[:])
            wih_s = cpool.tile([S + 1, 4 * H], F32)
            nc.sync.dma_start(wih_s[:], wih_p[:])
            whh_s = cpool.tile([H, 4 * H], F32)
            nc.sync.dma_start(whh_s[:], whh_p[:])
            wfc_s = cpool.tile([B, S], F32)
            nc.sync.dma_start(wfc_s[:], wfc_p[:])
            bfcrep_s = cpool.tile([128, S], F32)
            nc.sync.dma_start(bfcrep_s[:], bfcrep_p[:])

            xw_d = dpool.tile([N, S], F32)
            y_d = dpool.tile([N, S], F32)

            # --- Phase A: XW = x @ W^T (node-major in DRAM) ---
            with (
                tc.tile_pool(name="pa_sb", bufs=3) as pa,
                tc.tile_pool(name="pa_ps", bufs=3, space="PSUM") as pap,
            ):
                for t in range(NWIN):
                    r0 = t * 128
                    rows = min(128, N - r0)
                    xt = pa.tile([128, S], F32, tag="xt")
                    nc.sync.dma_start(xt[:rows, :], x_p[r0:r0 + rows, :])
                    psT = pap.tile([S, 128], F32, tag="psT")
                    nc.tensor.transpose(psT[:, :rows], xt[:rows, :],
                                        ident[:rows, :rows])
                    xTs = pa.tile([S, 128], F32, tag="xTs")
                    nc.vector.tensor_copy(xTs[:, :rows], psT[:, :rows])
                    psW = pap.tile([128, S], F32, tag="psW")
                    nc.tensor.matmul(psW[:rows, :], xTs[:, :rows], wt_s[:],
                                     start=True, stop=True)
                    xw_sb = pa.tile([128, S], F32, tag="xw_sb")
                    nc.vector.tensor_copy(xw_sb[:rows, :], psW[:rows, :])
                    nc.sync.dma_start(xw_d[r0:r0 + rows, :], xw_sb[:rows, :])

            if phase == "a":
                with tc.tile_pool(name="dbg", bufs=2) as dbg:
                    for t in range(NCHUNK_NODE):
                        dt_ = dbg.tile([128, S], F32, tag="d")
                        nc.sync.dma_start(dt_[:], xw_d[t * 128:(t + 1) * 128, :])
                        nc.sync.dma_start(out_p[t * 128:(t + 1) * 128, :], dt_[:])

            # --- Phase B: gather + one-hot scatter matmul ---
            if phase != "a":
                with (
                    tc.tile_pool(name="pb_g", bufs=4) as pg,
                    tc.tile_pool(name="pb_oh", bufs=4) as poh,
                    tc.tile_pool(name="pb_y", bufs=3) as py,
                    tc.tile_pool(name="pb_ps", bufs=4, space="PSUM") as pbp,
                ):
                    psum_w = None
                    for g in range(ngather):
                        gt = pg.tile([128, GCHUNK // 128, S], F32, tag="gt")
                        nc.gpsimd.dma_gather(
                            gt[:], xw_d[:, :],
                            gidx_s[:, g * idx_cols:(g + 1) * idx_cols],
                            GCHUNK, GCHUNK, S)
                        for j in range(GCHUNK // 128):
                            k = g * (GCHUNK // 128) + j
                            w = int(chunk_win[k])
                            first = k == 0 or int(chunk_win[k - 1]) != w
                            last = k == nch - 1 or int(chunk_win[k + 1]) != w
                            oh = poh.tile([128, 128], F32, tag="oh")
                            nc.vector.tensor_scalar(
                                oh[:], iota_row[:], off_s[:, k:k + 1],
                                nrm_s[:, k:k + 1],
                                op0=mybir.AluOpType.is_equal,
                                op1=mybir.AluOpType.mult)
                            if first:
                                psum_w = pbp.tile([128, S], F32, tag="pw")
                            nc.tensor.matmul(psum_w[:], oh[:], gt[:, j, :],
                                             start=first, stop=last)
                            if last:
                                r0 = w * WIN
                                rows = min(WIN, N - r0)
                                yt = py.tile([128, S], F32, tag="yt")
                                nc.vector.tensor_add(yt[:], psum_w[:], brep_s[:])
                                nc.sync.dma_start(y_d[r0:r0 + rows, :],
                                                  yt[:rows, :])

            if phase == "ab":
                with tc.tile_pool(name="dbg", bufs=2) as dbg:
                    for t in range(NCHUNK_NODE):
                        dt_ = dbg.tile([128, S], F32, tag="d")
                        nc.sync.dma_start(dt_[:], y_d[t * 128:(t + 1) * 128, :])
                        nc.sync.dma_start(out_p[t * 128:(t + 1) * 128, :], dt_[:])

            if phase in ("abr", "full"):
                # --- Reshard: AllToAll of x2^T node-slices ---
                a2a_send = dpool.tile([NCORES, S, NODES_PER_CORE], F32)
                a2a_recv = dpool.tile([NCORES, S, NODES_PER_CORE], F32)
                y_flat = y_d[:, :].rearrange("n s -> (n s)")
                with tc.tile_pool(name="rs", bufs=2) as rs:
                    for c in range(NCORES):
                        st = rs.tile([S, NODES_PER_CORE], F32, tag="st")
                        view = y_flat.rearrange("(a b) -> a b", b=N)
                        nc.sync.dma_start(
                            st[:],
                            view[:, c * NODES_PER_CORE:(c + 1) * NODES_PER_CORE])
                        nc.sync.dma_start(a2a_send[c], st[:])
                nc.gpsimd.collective_compute(
                    "AllToAll", mybir.AluOpType.bypass,
                    replica_groups=[list(range(NCORES))],
                    ins=[a2a_send.opt()], outs=[a2a_recv.opt()])

            if phase == "abr":
                with tc.tile_pool(name="dbg", bufs=2) as dbg:
                    flat_out = out_p[:, :].rearrange("n s -> (n s)")
                    flat_recv = a2a_recv.opt().rearrange("c s n -> (c s n)")
                    for t in range(8):
                        dt_ = dbg.tile([1, 10000], F32, tag="d")
                        nc.sync.dma_start(
                            dt_[:],
                            flat_recv[t * 10000:(t + 1) * 10000][None, :])
                        nc.sync.dma_start(
                            flat_out[t * 10000:(t + 1) * 10000][None, :],
                            dt_[:])

            if phase == "full":
                # --- Phase C: LSTM over 8 steps + FC, node-sharded ---
                with (
                    tc.tile_pool(name="pc_x", bufs=1) as pcx,
                    tc.tile_pool(name="pc_st", bufs=2) as pcs,
                    tc.tile_pool(name="pc_t", bufs=4) as pct,
                    tc.tile_pool(name="pc_ps", bufs=2, space="PSUM") as pcp,
                ):
                    x2a = pcx.tile([S + 1, B, NODE_PAD], F32)
                    nc.vector.memset(x2a[:, :, NODES_PER_CORE:], 0.0)
                    nc.vector.memset(x2a[S:S + 1, :, :NODES_PER_CORE], 1.0)
                    for b in range(B):
                        nc.sync.dma_start(x2a[:S, b, :NODES_PER_CORE],
                                          a2a_recv[b])

                    vstack = pcx.tile([128, NCHUNK_NODE, B], F32)

                    hT_prev = None
                    c_prev = None
                    for b in range(B):
                        hT = pcs.tile([H, NODE_PAD], F32, tag="hT")
                        c_cur = pcs.tile([128, NCHUNK_NODE, H], F32, tag="cc")
                        for t in range(NCHUNK_NODE):
                            sl = slice(t * 128, (t + 1) * 128)
                            ps_g = pcp.tile([128, 4 * H], F32, tag="psg")
                            nc.tensor.matmul(ps_g[:], x2a[:, b, sl], wih_s[:],
                                             start=True, stop=(b == 0))
                            if b > 0:
                                nc.tensor.matmul(ps_g[:], hT_prev[:, sl],
                                                 whh_s[:], start=False,
                                                 stop=True)
                            sif = pct.tile([128, 2 * H], F32, tag="sif")
                            nc.scalar.activation(
                                sif[:], ps_g[:, 0:2 * H],
                                mybir.ActivationFunctionType.Sigmoid,
                                bias=zb[:])
                            tg = pct.tile([128, H], F32, tag="tg")
                            nc.scalar.activation(
                                tg[:], ps_g[:, 2 * H:3 * H],
                                mybir.ActivationFunctionType.Tanh,
                                bias=zb[:])
                            so = pct.tile([128, H], F32, tag="so")
                            nc.scalar.activation(
                                so[:], ps_g[:, 3 * H:4 * H],
                                mybir.ActivationFunctionType.Sigmoid,
                                bias=zb[:])
                            if b == 0:
                                nc.vector.tensor_mul(c_cur[:, t, :],
                                                     sif[:, 0:H], tg[:])
                            else:
                                ig = pct.tile([128, H], F32, tag="ig")
                                nc.vector.tensor_mul(ig[:], sif[:, 0:H], tg[:])
                                fc = pct.tile([128, H], F32, tag="fc")
                                nc.vector.tensor_mul(fc[:], sif[:, H:2 * H],
                                                     c_prev[:, t, :])
                                nc.vector.tensor_add(c_cur[:, t, :], ig[:],
                                                     fc[:])
                            tc_t = pct.tile([128, H], F32, tag="tct")
                            nc.scalar.activation(
                                tc_t[:], c_cur[:, t, :],
                                mybir.ActivationFunctionType.Tanh,
                                bias=zb[:])
                            ht = pct.tile([128, H], F32, tag="ht")
                            nc.vector.tensor_mul(ht[:], so[:], tc_t[:])
                            ps_hT = pcp.tile([H, 128], F32, tag="psh")
                            nc.tensor.transpose(ps_hT[:], ht[:], ident[:])
                            nc.vector.tensor_copy(hT[:, sl], ps_hT[:])
                            nc.vector.tensor_copy(vstack[:, t, b:b + 1],
                                                  ht[:, H - 1:H])
                        hT_prev, c_prev = hT, c_cur

                    for t in range(NCHUNK_NODE):
                        sl = slice(t * 128, (t + 1) * 128)
                        ps_vT = pcp.tile([B, 128], F32, tag="psh")
                        nc.tensor.transpose(ps_vT[:], vstack[:, t, :], ident[:])
                        vt_c = pct.tile([B, 128], F32, tag="vtc")
                        nc.vector.tensor_copy(vt_c[:], ps_vT[:])
                        ps_fc = pcp.tile([128, S], F32, tag="psf")
                        nc.tensor.matmul(ps_fc[:], vt_c[:], wfc_s[:],
                                         start=True, stop=True)
                        ot = pct.tile([128, S], F32, tag="ot")
                        nc.vector.tensor_add(ot[:], ps_fc[:], bfcrep_s[:])
                        ot2 = pct.tile([128, S], F32, tag="ot2")
                        nc.vector.tensor_scalar_max(ot2[:], ot[:], 0.0)
                        nc.sync.dma_start(out_p[sl, :], ot2[:])

    nc.compile()
    return nc


def _prep_all(x, edge_index, edge_weight, W_gcn, b_gcn, Wih, Whh, bih, bhh,
              Wfc, bfc):
    gidx_w, off_p, nrm_p, chunk_win, total_g = _prep_edges(edge_index, edge_weight)
    nch = total_g // 128
    ngather = total_g // GCHUNK

    wt = np.ascontiguousarray(W_gcn.T.astype(np.float32))          # [s, o]
    brep = np.ascontiguousarray(
        np.broadcast_to(b_gcn.astype(np.float32), (128, S)))
    wih_aug = np.zeros((S + 1, 4 * H), np.float32)
    wih_aug[:S] = Wih.T
    wih_aug[S] = bih + bhh
    whh_t = np.ascontiguousarray(Whh.T.astype(np.float32))
    wfc_t = np.ascontiguousarray(Wfc.T.astype(np.float32))
    bfcrep = np.ascontiguousarray(
        np.broadcast_to(bfc.astype(np.float32), (128, S)))

    common = {"wt": wt, "brep": brep, "gidx": gidx_w, "offp": off_p,
              "nrmp": nrm_p, "wih": wih_aug, "whh": whh_t, "wfc": wfc_t,
              "bfcrep": bfcrep}
    in_maps = []
    for c in range(NCORES):
        m = dict(common)
        m["x"] = np.ascontiguousarray(x[c].astype(np.float32))
        in_maps.append(m)
    return in_maps, nch, ngather, chunk_win


def kernel(x, edge_index, edge_weight, W_gcn, b_gcn, Wih, Whh, bih, bhh,
           Wfc, bfc):
    x = np.asarray(x)
    in_maps, nch, ngather, chunk_win = _prep_all(
        np.asarray(x), np.asarray(edge_index), np.asarray(edge_weight),
        np.asarray(W_gcn), np.asarray(b_gcn), np.asarray(Wih),
        np.asarray(Whh), np.asarray(bih), np.asarray(bhh),
        np.asarray(Wfc), np.asarray(bfc))

    key = (nch, ngather, chunk_win.tobytes())
    if key not in _CACHE:
        _CACHE.clear()
        _CACHE[key] = _build(nch, ngather, chunk_win)
    nc = _CACHE[key]

    res = run_bass_kernel_spmd(nc, in_maps, list(range(NCORES)))
    full = np.empty((N, S), np.float32)
    for c in range(NCORES):
        full[c * NODES_PER_CORE:(c + 1) * NODES_PER_CORE] = \
            res.results[c]["out"][:NODES_PER_CORE]
    return np.ascontiguousarray(np.broadcast_to(full[None], (B, N, S)))


# revision 21
# speedup vs baseline: 1.0198x; 1.0198x over previous
"""GCN-LSTM fused kernel for Trainium2, 8 NeuronCores.

Strategy (sharding):
  Phase A+B (GCN conv): batch-sharded. Core b owns batch b. It computes
  XW = x[b] @ W^T into its DRAM, then processes ALL edges for its batch:
  dma_gather of XW rows by edge src (edge-major layout), a one-hot
  scatter matrix per 128-edge chunk built on DVE (iota==offset)*norm,
  and a PE matmul accumulating messages per 128-dst-node PSUM window
  (edges are pre-sorted by dst on host; self-loops are folded in as
  extra edges with weight dis^2). No cross-core traffic at all.
  Phase reshard: x2 = memory-reinterpretation of y ([64,10000] flat
  view). AllToAll moves node-slices so each core gets its 1250 nodes
  for every timestep.
  Phase C (LSTM over B=8 steps + FC): node-sharded, core c owns nodes
  [1250c, 1250c+1250). Gates via PE matmuls (bias folded via ones-row),
  nonlinearities on ACT, elementwise on DVE node-major, h transposed
  each step on PE for the recurrence.

Output y[b,n,:] is identical across b (reference broadcasts), so each
core returns its [1250, 64] node rows; host assembles + broadcasts.
"""

import numpy as np

import concourse.bass as bass
import concourse.bacc as bacc
import concourse.mybir as mybir
import concourse.tile as tile
from concourse.bass_utils import run_bass_kernel_spmd

F32 = mybir.dt.float32
I16 = mybir.dt.int16

B, N, S, H, E = 8, 10000, 64, 8, 160000
NCORES = 8
WIN = 128                      # dst-node window (psum partition dim)
NWIN = (N + WIN - 1) // WIN    # 79
GCHUNK = 1024                  # gather idxs per dma_gather (ucode ring cap)
NODES_PER_CORE = N // NCORES   # 1250
NODE_PAD = 1280                # padded to 10 chunks of 128
NCHUNK_NODE = NODE_PAD // 128  # 10

_CACHE = {}


def _prep_edges(edge_index, edge_weight):
    """Sort edges (+self-loops) by dst, pad to the chunk grid.

    Returns gather idxs (wrapped int16), per-chunk-col offsets/norms
    ([128, nch] f32), per-chunk window id, and dis for the host. All
    fp32 math here is O(E) index/scalar prep; the O(E*S) work is on
    device.
    """
    src = edge_index[0].astype(np.int64)
    dst = edge_index[1].astype(np.int64)
    ew = edge_weight.astype(np.float32)

    deg = np.zeros(N, np.float32)
    np.add.at(deg, dst, ew)
    deg += np.float32(1.0)
    dis = (1.0 / np.sqrt(deg)).astype(np.float32)
    norm = (dis[src] * dis[dst] * ew).astype(np.float32)

    # self loops: weight dis^2 at (n, n)
    s_all = np.concatenate([src, np.arange(N, dtype=np.int64)])
    d_all = np.concatenate([dst, np.arange(N, dtype=np.int64)])
    n_all = np.concatenate([norm, (dis * dis).astype(np.float32)])

    order = np.argsort(d_all, kind="stable")
    s_all, d_all, n_all = s_all[order], d_all[order], n_all[order]

    win = d_all // WIN
    counts = np.bincount(win, minlength=NWIN)
    padded = ((counts + 127) // 128) * 128
    total = int(padded.sum())
    total_g = ((total + GCHUNK - 1) // GCHUNK) * GCHUNK

    g_idx = np.zeros(total_g, np.int16)
    off = np.zeros(total_g, np.float32)
    nrm = np.zeros(total_g, np.float32)
    # scatter sorted edges into padded slots
    starts = np.zeros(NWIN + 1, np.int64)
    starts[1:] = np.cumsum(padded)
    csrc = np.zeros(NWIN + 1, np.int64)
    csrc[1:] = np.cumsum(counts)
    pos = starts[win] + (np.arange(len(d_all)) - csrc[win])
    g_idx[pos] = s_all.astype(np.int16)
    off[pos] = (d_all - win * WIN).astype(np.float32)
    nrm[pos] = n_all

    nch = total_g // 128
    chunk_win = np.minimum(starts[1:].searchsorted(np.arange(nch) * 128, side="right"),
                           NWIN - 1)
    # wrapped idx layout: [16, n/16] replicated to 128 partitions
    w = g_idx.reshape(-1, 16).T.astype(np.int16)
    gidx_w = np.tile(w, (8, 1))
    off_p = np.ascontiguousarray(off.reshape(nch, 128).T)
    nrm_p = np.ascontiguousarray(nrm.reshape(nch, 128).T)
    return gidx_w, off_p, nrm_p, chunk_win.astype(np.int64), total_g


def _build(nch, ngather, chunk_win):
    """Build the SPMD Bass/Tile program (identical on all cores)."""
    import os
    phase = os.environ.get("KPHASE", "full")  # a | ab | abr | full
    nc = bacc.Bacc("TRN2", target_bir_lowering=False)

    x_p = nc.declare_dram_parameter("x", [N, S], F32, isOutput=False)
    wt_p = nc.declare_dram_parameter("wt", [S, S], F32, isOutput=False)
    brep_p = nc.declare_dram_parameter("brep", [128, S], F32, isOutput=False)
    gidx_p = nc.declare_dram_parameter("gidx", [128, ngather * GCHUNK // 16],
                                       I16, isOutput=False)
    off_p = nc.declare_dram_parameter("offp", [128, nch], F32, isOutput=False)
    nrm_p = nc.declare_dram_parameter("nrmp", [128, nch], F32, isOutput=False)
    wih_p = nc.declare_dram_parameter("wih", [S + 1, 4 * H], F32, isOutput=False)
    whh_p = nc.declare_dram_parameter("whh", [H, 4 * H], F32, isOutput=False)
    wfc_p = nc.declare_dram_parameter("wfc", [B, S], F32, isOutput=False)
    bfcrep_p = nc.declare_dram_parameter("bfcrep", [128, S], F32, isOutput=False)
    out_p = nc.declare_dram_parameter("out", [NODE_PAD, S], F32, isOutput=True)

    idx_cols = GCHUNK // 16

    with tile.TileContext(nc) as tc:
        with (
            tc.tile_pool(name="const", bufs=1) as cpool,
            tc.tile_pool(name="dram", bufs=1, space="DRAM") as dpool,
        ):
            # --- constants ---
            iota_row = cpool.tile([128, 128], F32)
            nc.gpsimd.iota(iota_row[:], pattern=[[1, 128]], base=0,
                           channel_multiplier=0,
                           allow_small_or_imprecise_dtypes=True)
            iota_col = cpool.tile([128, 1], F32)
            nc.gpsimd.iota(iota_col[:], pattern=[[1, 1]], base=0,
                           channel_multiplier=1,
                           allow_small_or_imprecise_dtypes=True)
            ident = cpool.tile([128, 128], F32)
            nc.vector.tensor_scalar(ident[:], iota_row[:], iota_col[:], None,
                                    op0=mybir.AluOpType.is_equal)
            zb = cpool.tile([128, 1], F32)
            nc.vector.memset(zb[:], 0.0)
            wt_s = cpool.tile([S, S], F32)
            nc.sync.dma_start(wt_s[:], wt_p[:])
            brep_s = cpool.tile([128, S], F32)
            nc.sync.dma_start(brep_s[:], brep_p[:])
            gidx_s = cpool.tile([128, ngather * idx_cols], I16)
            nc.sync.dma_start(gidx_s[:], gidx_p[:])
            off_s = cpool.tile([128, nch], F32)
            nc.sync.dma_start(off_s[:], off_p[:])
            nrm_s = cpool.tile([128, nch], F32)
            nc.sync.dma_start(nrm_s[:], nrm_p[:])
            wih_s = cpool.tile([S + 1, 4 * H], F32)
            nc.sync.dma_start(wih_s[:], wih_p[:])
            whh_s = cpool.tile([H, 4 * H], F32)
            nc.sync.dma_start(whh_s[:], whh_p[:])
            wfc_s = cpool.tile([B, S], F32)
            nc.sync.dma_start(wfc_s[:], wfc_p[:])
            bfcrep_s = cpool.tile([128, S], F32)
            nc.sync.dma_start(bfcrep_s[:], bfcrep_p[:])

            xw_d = dpool.tile([N, S], F32)
            y_d = dpool.tile([N, S], F32)

            # --- Phase A: XW = x @ W^T (node-major in DRAM) ---
            with (
                tc.tile_pool(name="pa_sb", bufs=3) as pa,
                tc.tile_pool(name="pa_ps", bufs=3, space="PSUM") as pap,
            ):
                for t in range(NWIN):
                    r0 = t * 128
                    rows = min(128, N - r0)
                    xt = pa.tile([128, S], F32, tag="xt")
                    nc.sync.dma_start(xt[:rows, :], x_p[r0:r0 + rows, :])
                    psT = pap.tile([S, 128], F32, tag="psT")
                    nc.tensor.transpose(psT[:, :rows], xt[:rows, :],
                                        ident[:rows, :rows])
                    xTs = pa.tile([S, 128], F32, tag="xTs")
                    nc.scalar.copy(xTs[:, :rows], psT[:, :rows])
                    psW = pap.tile([128, S], F32, tag="psW")
                    nc.tensor.matmul(psW[:rows, :], xTs[:, :rows], wt_s[:],
                                     start=True, stop=True)
                    xw_sb = pa.tile([128, S], F32, tag="xw_sb")
                    nc.vector.tensor_copy(xw_sb[:rows, :], psW[:rows, :])
                    nc.sync.dma_start(xw_d[r0:r0 + rows, :], xw_sb[:rows, :])

            if phase == "a":
                with tc.tile_pool(name="dbg", bufs=2) as dbg:
                    for t in range(NCHUNK_NODE):
                        dt_ = dbg.tile([128, S], F32, tag="d")
                        nc.sync.dma_start(dt_[:], xw_d[t * 128:(t + 1) * 128, :])
                        nc.sync.dma_start(out_p[t * 128:(t + 1) * 128, :], dt_[:])

            # --- Phase B: gather + one-hot scatter matmul ---
            if phase != "a":
                with (
                    tc.tile_pool(name="pb_g", bufs=4) as pg,
                    tc.tile_pool(name="pb_oh", bufs=4) as poh,
                    tc.tile_pool(name="pb_y", bufs=3) as py,
                    tc.tile_pool(name="pb_ps", bufs=4, space="PSUM") as pbp,
                ):
                    psum_w = None
                    for g in range(ngather):
                        gt = pg.tile([128, GCHUNK // 128, S], F32, tag="gt")
                        nc.gpsimd.dma_gather(
                            gt[:], xw_d[:, :],
                            gidx_s[:, g * idx_cols:(g + 1) * idx_cols],
                            GCHUNK, GCHUNK, S)
                        for j in range(GCHUNK // 128):
                            k = g * (GCHUNK // 128) + j
                            w = int(chunk_win[k])
                            first = k == 0 or int(chunk_win[k - 1]) != w
                            last = k == nch - 1 or int(chunk_win[k + 1]) != w
                            oh = poh.tile([128, 128], F32, tag="oh")
                            nc.vector.tensor_scalar(
                                oh[:], iota_row[:], off_s[:, k:k + 1],
                                nrm_s[:, k:k + 1],
                                op0=mybir.AluOpType.is_equal,
                                op1=mybir.AluOpType.mult)
                            if first:
                                psum_w = pbp.tile([128, S], F32, tag="pw")
                            nc.tensor.matmul(psum_w[:], oh[:], gt[:, j, :],
                                             start=first, stop=last)
                            if last:
                                r0 = w * WIN
                                rows = min(WIN, N - r0)
                                yt = py.tile([128, S], F32, tag="yt")
                                nc.vector.tensor_add(yt[:], psum_w[:], brep_s[:])
                                nc.sync.dma_start(y_d[r0:r0 + rows, :],
                                                  yt[:rows, :])

            if phase == "ab":
                with tc.tile_pool(name="dbg", bufs=2) as dbg:
                    for t in range(NCHUNK_NODE):
                        dt_ = dbg.tile([128, S], F32, tag="d")
                        nc.sync.dma_start(dt_[:], y_d[t * 128:(t + 1) * 128, :])
                        nc.sync.dma_start(out_p[t * 128:(t + 1) * 128, :], dt_[:])

            if phase in ("abr", "full"):
                # --- Reshard: AllToAll of x2^T node-slices ---
                a2a_send = dpool.tile([NCORES, S, NODES_PER_CORE], F32)
                a2a_recv = dpool.tile([NCORES, S, NODES_PER_CORE], F32)
                y_flat = y_d[:, :].rearrange("n s -> (n s)")
                with tc.tile_pool(name="rs", bufs=2) as rs:
                    for c in range(NCORES):
                        st = rs.tile([S, NODES_PER_CORE], F32, tag="st")
                        view = y_flat.rearrange("(a b) -> a b", b=N)
                        nc.sync.dma_start(
                            st[:],
                            view[:, c * NODES_PER_CORE:(c + 1) * NODES_PER_CORE])
                        nc.sync.dma_start(a2a_send[c], st[:])
                nc.gpsimd.collective_compute(
                    "AllToAll", mybir.AluOpType.bypass,
                    replica_groups=[list(range(NCORES))],
                    ins=[a2a_send.opt()], outs=[a2a_recv.opt()])

            if phase == "abr":
                with tc.tile_pool(name="dbg", bufs=2) as dbg:
                    flat_out = out_p[:, :].rearrange("n s -> (n s)")
                    flat_recv = a2a_recv.opt().rearrange("c s n -> (c s n)")
                    for t in range(8):
                        dt_ = dbg.tile([1, 10000], F32, tag="d")
                        nc.sync.dma_start(
                            dt_[:],
                            flat_recv[t * 10000:(t + 1) * 10000][None, :])
                        nc.sync.dma_start(
                            flat_out[t * 10000:(t + 1) * 10000][None, :],
                            dt_[:])

            if phase == "full":
                # --- Phase C: LSTM over 8 steps + FC, node-sharded ---
                with (
                    tc.tile_pool(name="pc_x", bufs=1) as pcx,
                    tc.tile_pool(name="pc_st", bufs=2) as pcs,
                    tc.tile_pool(name="pc_t", bufs=4) as pct,
                    tc.tile_pool(name="pc_ps", bufs=2, space="PSUM") as pcp,
                ):
                    x2a = pcx.tile([S + 1, B, NODE_PAD], F32)
                    nc.vector.memset(x2a[:, :, NODES_PER_CORE:], 0.0)
                    nc.vector.memset(x2a[S:S + 1, :, :NODES_PER_CORE], 1.0)
                    for b in range(B):
                        nc.sync.dma_start(x2a[:S, b, :NODES_PER_CORE],
                                          a2a_recv[b])

                    vstack = pcx.tile([128, NCHUNK_NODE, B], F32)

                    hT_prev = None
                    c_prev = None
                    for b in range(B):
                        hT = pcs.tile([H, NODE_PAD], F32, tag="hT")
                        c_cur = pcs.tile([128, NCHUNK_NODE, H], F32, tag="cc")
                        for t in range(NCHUNK_NODE):
                            sl = slice(t * 128, (t + 1) * 128)
                            ps_g = pcp.tile([128, 4 * H], F32, tag="psg")
                            nc.tensor.matmul(ps_g[:], x2a[:, b, sl], wih_s[:],
                                             start=True, stop=(b == 0))
                            if b > 0:
                                nc.tensor.matmul(ps_g[:], hT_prev[:, sl],
                                                 whh_s[:], start=False,
                                                 stop=True)
                            # gates packed (i, f, o, g): one sigmoid covers i,f,o
                            sig = pct.tile([128, 3 * H], F32, tag="sig")
                            nc.scalar.activation(
                                sig[:], ps_g[:, 0:3 * H],
                                mybir.ActivationFunctionType.Sigmoid,
                                bias=zb[:])
                            tg = pct.tile([128, H], F32, tag="tg")
                            nc.scalar.activation(
                                tg[:], ps_g[:, 3 * H:4 * H],
                                mybir.ActivationFunctionType.Tanh,
                                bias=zb[:])
                            if b == 0:
                                nc.vector.tensor_mul(c_cur[:, t, :],
                                                     sig[:, 0:H], tg[:])
                            else:
                                ig = pct.tile([128, H], F32, tag="ig")
                                nc.vector.tensor_mul(ig[:], sig[:, 0:H], tg[:])
                                fc = pct.tile([128, H], F32, tag="fc")
                                nc.vector.tensor_mul(fc[:], sig[:, H:2 * H],
                                                     c_prev[:, t, :])
                                nc.vector.tensor_add(c_cur[:, t, :], ig[:],
                                                     fc[:])
                            tc_t = pct.tile([128, H], F32, tag="tct")
                            nc.scalar.activation(
                                tc_t[:], c_cur[:, t, :],
                                mybir.ActivationFunctionType.Tanh,
                                bias=zb[:])
                            ht = pct.tile([128, H], F32, tag="ht")
                            nc.vector.tensor_mul(ht[:], sig[:, 2 * H:3 * H],
                                                 tc_t[:])
                            ps_hT = pcp.tile([H, 128], F32, tag="psh")
                            nc.tensor.transpose(ps_hT[:], ht[:], ident[:])
                            nc.vector.tensor_copy(hT[:, sl], ps_hT[:])
                            nc.vector.tensor_copy(vstack[:, t, b:b + 1],
                                                  ht[:, H - 1:H])
                        hT_prev, c_prev = hT, c_cur

                    for t in range(NCHUNK_NODE):
                        sl = slice(t * 128, (t + 1) * 128)
                        ps_vT = pcp.tile([B, 128], F32, tag="psh")
                        nc.tensor.transpose(ps_vT[:], vstack[:, t, :], ident[:])
                        vt_c = pct.tile([B, 128], F32, tag="vtc")
                        nc.vector.tensor_copy(vt_c[:], ps_vT[:])
                        ps_fc = pcp.tile([128, S], F32, tag="psf")
                        nc.tensor.matmul(ps_fc[:], vt_c[:], wfc_s[:],
                                         start=True, stop=True)
                        ot = pct.tile([128, S], F32, tag="ot")
                        nc.vector.tensor_add(ot[:], ps_fc[:], bfcrep_s[:])
                        ot2 = pct.tile([128, S], F32, tag="ot2")
                        nc.vector.tensor_scalar_max(ot2[:], ot[:], 0.0)
                        nc.sync.dma_start(out_p[sl, :], ot2[:])

    nc.compile()
    return nc


def _prep_all(x, edge_index, edge_weight, W_gcn, b_gcn, Wih, Whh, bih, bhh,
              Wfc, bfc):
    gidx_w, off_p, nrm_p, chunk_win, total_g = _prep_edges(edge_index, edge_weight)
    nch = total_g // 128
    ngather = total_g // GCHUNK

    wt = np.ascontiguousarray(W_gcn.T.astype(np.float32))          # [s, o]
    brep = np.ascontiguousarray(
        np.broadcast_to(b_gcn.astype(np.float32), (128, S)))
    # repack torch gate order (i, f, g, o) -> (i, f, o, g)
    perm = np.r_[0:2 * H, 3 * H:4 * H, 2 * H:3 * H]
    wih_aug = np.zeros((S + 1, 4 * H), np.float32)
    wih_aug[:S] = Wih.T[:, perm]
    wih_aug[S] = (bih + bhh)[perm]
    whh_t = np.ascontiguousarray(Whh.T[:, perm].astype(np.float32))
    wfc_t = np.ascontiguousarray(Wfc.T.astype(np.float32))
    bfcrep = np.ascontiguousarray(
        np.broadcast_to(bfc.astype(np.float32), (128, S)))

    common = {"wt": wt, "brep": brep, "gidx": gidx_w, "offp": off_p,
              "nrmp": nrm_p, "wih": wih_aug, "whh": whh_t, "wfc": wfc_t,
              "bfcrep": bfcrep}
    in_maps = []
    for c in range(NCORES):
        m = dict(common)
        m["x"] = np.ascontiguousarray(x[c].astype(np.float32))
        in_maps.append(m)
    return in_maps, nch, ngather, chunk_win


def kernel(x, edge_index, edge_weight, W_gcn, b_gcn, Wih, Whh, bih, bhh,
           Wfc, bfc):
    x = np.asarray(x)
    in_maps, nch, ngather, chunk_win = _prep_all(
        np.asarray(x), np.asarray(edge_index), np.asarray(edge_weight),
        np.asarray(W_gcn), np.asarray(b_gcn), np.asarray(Wih),
        np.asarray(Whh), np.asarray(bih), np.asarray(bhh),
        np.asarray(Wfc), np.asarray(bfc))

    key = (nch, ngather, chunk_win.tobytes())
    if key not in _CACHE:
        _CACHE.clear()
        _CACHE[key] = _build(nch, ngather, chunk_win)
    nc = _CACHE[key]

    res = run_bass_kernel_spmd(nc, in_maps, list(range(NCORES)))
    full = np.empty((N, S), np.float32)
    for c in range(NCORES):
        full[c * NODES_PER_CORE:(c + 1) * NODES_PER_CORE] = \
            res.results[c]["out"][:NODES_PER_CORE]
    return np.ascontiguousarray(np.broadcast_to(full[None], (B, N, S)))
